# revision 9
# baseline (speedup 1.0000x reference)
"""Trainium2 Bass kernel for nn_CEABlock (attention + candidate elimination + MLP).

Data-parallel over batch: 32 batch rows x 2 streams (x, xi) = 64 sequences,
8 per NeuronCore.  Each sequence [320, 768] runs: LN1 -> QKV -> attention
(probs are an output) -> top-k candidate elimination via on-device argsort
(rank = pairwise-compare counts, gathers via one-hot matmul) -> LN2 -> MLP.

Precision: the top-k ordering is driven by attn_t (mean of softmax columns),
whose adjacent sorted gaps sit at the fp32 ULP level, so the path that feeds
it (LN1, Q(first 64 tokens), K, scores, softmax) runs in true fp32.
Everything else (V, AV, proj, MLP, gathers) runs in float32r (13-bit
mantissa, 4x faster on the PE), well inside output tolerance.
"""

import numpy as np

B = 32
L = 320
DIM = 768
HEADS = 12
HD = 64
LT = 64
LS = 256
KEEP = 180
LOUT = LT + KEEP  # 244
HID = 3072
NCORES = 8
BSH = B // NCORES        # batch rows per core
NSB = BSH * 2            # sequences per core (x then xi)
EPS = 1e-5

_CACHE = {}

TCH = ((0, 128), (128, 128), (256, 64))   # token chunks of 320
OCH = ((0, 128), (128, 116))              # token chunks of 244


def _build_program():
    import concourse.bacc as bacc
    import concourse.mybir as mybir
    from concourse.tile import TileContext
    from concourse.masks import make_identity

    F32 = mybir.dt.float32
    F32R = mybir.dt.float32r
    AF = mybir.ActivationFunctionType
    OP = mybir.AluOpType
    AX = mybir.AxisListType

    nc = bacc.Bacc("TRN2", target_bir_lowering=False, debug=False)

    xs_d = nc.dram_tensor("xs", [NSB, L, DIM], F32, kind="ExternalInput")
    g_d = nc.dram_tensor("gidx", [NSB, 2, 128, 8], F32, kind="ExternalInput")
    qkvw_d = nc.dram_tensor("qkv_w", [DIM, 3 * DIM], F32, kind="ExternalInput")
    qkvb_d = nc.dram_tensor("qkv_b", [1, 3 * DIM], F32, kind="ExternalInput")
    n1w_d = nc.dram_tensor("n1w", [1, DIM], F32, kind="ExternalInput")
    n1b_d = nc.dram_tensor("n1b", [1, DIM], F32, kind="ExternalInput")
    projw_d = nc.dram_tensor("proj_w", [DIM, DIM], F32, kind="ExternalInput")
    projb_d = nc.dram_tensor("proj_b", [1, DIM], F32, kind="ExternalInput")
    n2w_d = nc.dram_tensor("n2w", [1, DIM], F32, kind="ExternalInput")
    n2b_d = nc.dram_tensor("n2b", [1, DIM], F32, kind="ExternalInput")
    fc1w_d = nc.dram_tensor("fc1_w", [DIM, HID], F32, kind="ExternalInput")
    fc1b_d = nc.dram_tensor("fc1_b", [1, HID], F32, kind="ExternalInput")
    fc2w_d = nc.dram_tensor("fc2_w", [HID, DIM], F32, kind="ExternalInput")
    fc2b_d = nc.dram_tensor("fc2_b", [1, DIM], F32, kind="ExternalInput")

    attn_d = nc.dram_tensor("attn", [NSB, HEADS, L, L], F32, kind="ExternalOutput")
    xo_d = nc.dram_tensor("xo", [NSB, LOUT, DIM], F32, kind="ExternalOutput")
    sg_d = nc.dram_tensor("sg", [NSB, 2, 128, 1], F32, kind="ExternalOutput")

    SCL = 0.125          # HD ** -0.5
    INV768 = float(np.float32(1.0 / 768.0))

    with TileContext(nc) as tc:
        with (
            tc.tile_pool(name="consts", bufs=1) as cp,
            tc.tile_pool(name="spill", bufs=1, space="DRAM") as dp,
        ):
            xsp_d = dp.tile([NSB, LOUT, DIM], F32)

            ident = cp.tile([128, 128], F32)
            make_identity(nc, ident)
            iota_f = cp.tile([128, 256], F32)
            nc.gpsimd.iota(iota_f, pattern=[[1, 256]], base=0, channel_multiplier=0,
                           allow_small_or_imprecise_dtypes=True)
            ones64 = cp.tile([64, 1], F32)
            nc.vector.memset(ones64, 1.0)
            eps128 = cp.tile([128, 1], F32)
            nc.vector.memset(eps128, EPS)
            n1w_c = cp.tile([128, 6, 1], F32)
            nc.sync.dma_start(out=n1w_c, in_=n1w_d.ap().rearrange("o (c p) -> p c o", p=128))
            n1b_c = cp.tile([128, 6, 1], F32)
            nc.sync.dma_start(out=n1b_c, in_=n1b_d.ap().rearrange("o (c p) -> p c o", p=128))
            n2w_c = cp.tile([128, 6, 1], F32)
            nc.sync.dma_start(out=n2w_c, in_=n2w_d.ap().rearrange("o (c p) -> p c o", p=128))
            n2b_c = cp.tile([128, 6, 1], F32)
            nc.sync.dma_start(out=n2b_c, in_=n2b_d.ap().rearrange("o (c p) -> p c o", p=128))

            # ================= phase A: attention + candidate elimination =====
            with tc.tile_pool(name="wa", bufs=1) as wa:
                Wq = wa.tile([128, 6, DIM], F32)
                Wk = wa.tile([128, 6, DIM], F32)
                Wq_r = wa.tile([128, 6, DIM], F32R)
                Wv_r = wa.tile([128, 6, DIM], F32R)
                proj_r = wa.tile([128, 6, DIM], F32R)
                projb_rep = wa.tile([128, DIM], F32)
                bv_rep = wa.tile([128, DIM], F32)
                bq_col = wa.tile([128, 6, 1], F32)
                bk_col = wa.tile([128, 6, 1], F32)

                with (
                    tc.tile_pool(name="wtmp", bufs=2) as wt_pool,
                    tc.tile_pool(name="ppW", bufs=1, space="PSUM") as ppW,
                ):
                    qkvb_row = wt_pool.tile([1, 3 * DIM], F32, tag="brow", bufs=1)
                    nc.sync.dma_start(out=qkvb_row, in_=qkvb_d.ap())
                    pb_row = wt_pool.tile([1, DIM], F32, tag="prow", bufs=1)
                    nc.sync.dma_start(out=pb_row, in_=projb_d.ap())
                    nc.gpsimd.partition_broadcast(projb_rep, pb_row)

                    # bias_eff = qkv_b + n1b @ (diag(n1w) qkv_w)
                    pbias = [ppW.tile([1, 384], F32, tag="pbias", bufs=6, name=f"pbias{i}")
                             for i in range(6)]
                    for c in range(6):
                        wt = wt_pool.tile([128, 3 * DIM], F32, tag="wt")
                        nc.sync.dma_start(out=wt, in_=qkvw_d.ap()[c * 128:(c + 1) * 128, :])
                        vtmp = wt_pool.tile([128, DIM], F32, tag="vt")
                        nc.vector.tensor_scalar(out=Wq[:, c, :], in0=wt[:, 0:DIM],
                                                scalar1=n1w_c[:, c, :], scalar2=None,
                                                op0=OP.mult)
                        nc.vector.tensor_scalar(out=Wk[:, c, :], in0=wt[:, DIM:2 * DIM],
                                                scalar1=n1w_c[:, c, :], scalar2=None,
                                                op0=OP.mult)
                        nc.vector.tensor_scalar(out=vtmp, in0=wt[:, 2 * DIM:3 * DIM],
                                                scalar1=n1w_c[:, c, :], scalar2=None,
                                                op0=OP.mult)
                        nc.vector.tensor_copy(out=Wq_r[:, c, :], in_=Wq[:, c, :])
                        nc.vector.tensor_copy(out=Wv_r[:, c, :], in_=vtmp)
                        for nn in range(2):
                            nc.tensor.matmul(pbias[0 + nn], n1b_c[:, c, :],
                                             Wq[:, c, nn * 384:(nn + 1) * 384],
                                             start=(c == 0), stop=(c == 5))
                            nc.tensor.matmul(pbias[2 + nn], n1b_c[:, c, :],
                                             Wk[:, c, nn * 384:(nn + 1) * 384],
                                             start=(c == 0), stop=(c == 5))
                            nc.tensor.matmul(pbias[4 + nn], n1b_c[:, c, :],
                                             vtmp[:, nn * 384:(nn + 1) * 384],
                                             start=(c == 0), stop=(c == 5))
                        pt = wt_pool.tile([128, DIM], F32, tag="pw")
                        nc.sync.dma_start(out=pt,
                                          in_=projw_d.ap()[c * 128:(c + 1) * 128, :])
                        nc.vector.tensor_copy(out=proj_r[:, c, :], in_=pt)
                    bqkv_row = wt_pool.tile([1, 3 * DIM], F32, tag="berow", bufs=1)
                    for i in range(6):
                        nc.vector.tensor_tensor(out=bqkv_row[:, i * 384:(i + 1) * 384],
                                                in0=pbias[i],
                                                in1=qkvb_row[:, i * 384:(i + 1) * 384],
                                                op=OP.add)
                    nc.gpsimd.partition_broadcast(bv_rep, bqkv_row[:, 2 * DIM:3 * DIM])
                    for c in range(6):
                        for dst, off in ((bq_col, 0), (bk_col, DIM)):
                            pb = ppW.tile([128, 1], F32, tag="pbt", bufs=2)
                            nc.tensor.transpose(
                                pb, bqkv_row[:, off + c * 128:off + (c + 1) * 128],
                                ident[:1, :1])
                            nc.vector.tensor_copy(out=dst[:, c, :], in_=pb)

                with (
                    tc.tile_pool(name="pa", bufs=1) as pa,
                    tc.tile_pool(name="pah", bufs=2) as pah,
                    tc.tile_pool(name="ppA", bufs=1, space="PSUM") as ppA,
                ):
                    for sb in range(NSB):
                        # ---- load + LN1 ----
                        xt = pa.tile([128, 3, DIM], F32, tag="xt")
                        for c, (t0, tsz) in enumerate(TCH):
                            nc.sync.dma_start(out=xt[:tsz, c, :],
                                              in_=xs_d.ap()[sb, t0:t0 + tsz, :])
                        ln = pa.tile([128, 3, DIM], F32, tag="ln")
                        for c, (t0, tsz) in enumerate(TCH):
                            st = pah.tile([128, 3, 6], F32, tag="bnst")
                            for g in range(3):
                                nc.vector.bn_stats(out=st[:tsz, g, :],
                                                   in_=xt[:tsz, c, g * 256:(g + 1) * 256])
                            mv = pah.tile([128, 2], F32, tag="bnmv")
                            nc.vector.bn_aggr(out=mv[:tsz], in_=st[:tsz])
                            # rstd = 1/sqrt(var+eps), Newton-refined
                            sd = pah.tile([128, 4], F32, tag="sd")
                            nc.scalar.activation(out=sd[:tsz, 0:1], in_=mv[:tsz, 1:2],
                                                 func=AF.Sqrt, bias=eps128[:tsz], scale=1.0)
                            nc.vector.reciprocal(out=sd[:tsz, 1:2], in_=sd[:tsz, 0:1])
                            nc.vector.tensor_tensor(out=sd[:tsz, 2:3], in0=sd[:tsz, 1:2],
                                                    in1=sd[:tsz, 1:2], op=OP.mult)
                            nc.vector.tensor_scalar(out=sd[:tsz, 3:4], in0=mv[:tsz, 1:2],
                                                    scalar1=eps128[:tsz], scalar2=None,
                                                    op0=OP.add)
                            nc.vector.tensor_tensor(out=sd[:tsz, 2:3], in0=sd[:tsz, 2:3],
                                                    in1=sd[:tsz, 3:4], op=OP.mult)
                            nc.vector.tensor_scalar(out=sd[:tsz, 2:3], in0=sd[:tsz, 2:3],
                                                    scalar1=-0.5, scalar2=1.5,
                                                    op0=OP.mult, op1=OP.add)
                            nc.vector.tensor_tensor(out=sd[:tsz, 1:2], in0=sd[:tsz, 1:2],
                                                    in1=sd[:tsz, 2:3], op=OP.mult)
                            nc.vector.tensor_scalar(out=ln[:tsz, c, :], in0=xt[:tsz, c, :],
                                                    scalar1=mv[:tsz, 0:1],
                                                    scalar2=sd[:tsz, 1:2],
                                                    op0=OP.subtract, op1=OP.mult)
                        # ---- lnT (PE transpose) ----
                        lnT = pa.tile([128, 6, L], F32, tag="lnT")
                        lnT_r = pa.tile([128, 6, L], F32R, tag="lnTr")
                        for dc in range(6):
                            pt = ppA.tile([128, L], F32, tag="ps", bufs=2)
                            for c, (t0, tsz) in enumerate(TCH):
                                nc.tensor.matmul(pt[:, t0:t0 + tsz],
                                                 ln[:tsz, c, dc * 128:(dc + 1) * 128],
                                                 ident[:tsz, :tsz], is_transpose=True,
                                                 skip_group_check=True)
                            nc.scalar.copy(out=lnT[:, dc, :], in_=pt)
                            nc.vector.tensor_copy(out=lnT_r[:, dc, :], in_=pt)
                        # ---- V (fp32r, [t, f] layout) ----
                        V_r = pa.tile([128, 3, DIM], F32R, tag="Vr_xpa")
                        for c, (t0, tsz) in enumerate(TCH):
                            for nn in range(2):
                                pv = ppA.tile([128, 384], F32, tag="psv", bufs=2)
                                for kc in range(6):
                                    nc.tensor.matmul(pv[:tsz, :],
                                                     lnT_r[:, kc, t0:t0 + tsz],
                                                     Wv_r[:, kc, nn * 384:(nn + 1) * 384],
                                                     start=(kc == 0), stop=(kc == 5))
                                nc.vector.tensor_tensor(
                                    out=V_r[:tsz, c, nn * 384:(nn + 1) * 384],
                                    in0=pv[:tsz, :],
                                    in1=bv_rep[:tsz, nn * 384:(nn + 1) * 384], op=OP.add)
                        # ---- head loop ----
                        l64 = pa.tile([64, 12], F32, tag="l64")
                        il64 = pa.tile([64, 12], F32, tag="il64")
                        l_r = pa.tile([128, 2, 12], F32, tag="l_r")
                        il_r = pa.tile([128, 2, 12], F32, tag="il_r")
                        acc = pa.tile([64, 256], F32, tag="acc")
                        avT_r = pa.tile([128, 6, L], F32R, tag="avT")
                        for hc in range(6):
                            kT_c = pah.tile([128, L], F32, tag="kTc")
                            kTr_c = pah.tile([128, L], F32R, tag="kTrc")
                            pk = ppA.tile([128, L], F32, tag="ps", bufs=2)
                            for kc in range(6):
                                nc.tensor.matmul(pk, Wk[:, kc, hc * 128:(hc + 1) * 128],
                                                 lnT[:, kc, :], start=(kc == 0),
                                                 stop=(kc == 5))
                            nc.scalar.activation(out=kT_c, in_=pk, func=AF.Identity,
                                                 bias=bk_col[:, hc, :], scale=1.0)
                            nc.vector.tensor_copy(out=kTr_c, in_=kT_c)
                            qT64_c = pah.tile([128, 64], F32, tag="q64c")
                            pq = ppA.tile([128, L], F32, tag="ps", bufs=2)
                            for kc in range(6):
                                nc.tensor.matmul(pq[:, :64],
                                                 Wq[:, kc, hc * 128:(hc + 1) * 128],
                                                 lnT[:, kc, 0:64], start=(kc == 0),
                                                 stop=(kc == 5))
                            nc.scalar.activation(out=qT64_c, in_=pq[:, :64],
                                                 func=AF.Identity,
                                                 bias=bq_col[:, hc, :], scale=1.0)
                            qTr_c = pah.tile([128, L], F32R, tag="qTrc")
                            pq2 = ppA.tile([128, L], F32, tag="ps", bufs=2)
                            for kc in range(6):
                                nc.tensor.matmul(pq2, Wq_r[:, kc, hc * 128:(hc + 1) * 128],
                                                 lnT_r[:, kc, :], start=(kc == 0),
                                                 stop=(kc == 5))
                            nc.scalar.activation(out=qTr_c, in_=pq2, func=AF.Identity,
                                                 bias=bq_col[:, hc, :], scale=1.0)

                            for hp in (0, 64):
                                h = hc * 2 + (hp // 64)
                                # fp32 ordering path
                                ps64 = ppA.tile([64, L], F32, tag="ps64", bufs=1)
                                nc.tensor.matmul(ps64, qT64_c[hp:hp + 64, :],
                                                 kT_c[hp:hp + 64, :], start=True, stop=True)
                                a64h = pah.tile([64, L], F32, tag="a64h")
                                nc.scalar.activation(out=a64h, in_=ps64, func=AF.Exp,
                                                     scale=SCL, accum_out=l64[:, h:h + 1])
                                nc.vector.reciprocal(out=il64[:, h:h + 1],
                                                     in_=l64[:, h:h + 1])
                                nc.vector.tensor_scalar(out=a64h, in0=a64h,
                                                        scalar1=il64[:, h:h + 1],
                                                        scalar2=None, op0=OP.mult)
                                nc.sync.dma_start(out=attn_d.ap()[sb, h, 0:64, :],
                                                  in_=a64h)
                                if h == 0:
                                    nc.vector.tensor_copy(out=acc, in_=a64h[:, 64:L])
                                else:
                                    nc.vector.tensor_tensor(out=acc, in0=acc,
                                                            in1=a64h[:, 64:L], op=OP.add)
                                # fp32r output path, q rows 64..320
                                for qc, q0 in enumerate((64, 192)):
                                    psr = ppA.tile([128, L], F32, tag="ps", bufs=2)
                                    nc.tensor.matmul(psr, qTr_c[hp:hp + 64, q0:q0 + 128],
                                                     kTr_c[hp:hp + 64, :],
                                                     start=True, stop=True)
                                    arn = pah.tile([128, L], F32, tag="arn")
                                    nc.scalar.activation(out=arn, in_=psr, func=AF.Exp,
                                                         scale=SCL,
                                                         accum_out=l_r[:, qc, h:h + 1])
                                    nc.vector.reciprocal(out=il_r[:, qc, h:h + 1],
                                                         in_=l_r[:, qc, h:h + 1])
                                    nc.vector.tensor_scalar(out=arn, in0=arn,
                                                            scalar1=il_r[:, qc, h:h + 1],
                                                            scalar2=None, op0=OP.mult)
                                    nc.sync.dma_start(
                                        out=attn_d.ap()[sb, h, q0:q0 + 128, :], in_=arn)
                                # S^T -> exp (unnormalized)
                                expT = pah.tile([128, 3, L], F32R, tag="expT", bufs=1)
                                for kc, (k0, ksz) in enumerate(TCH):
                                    pst = ppA.tile([128, L], F32, tag="ps", bufs=2)
                                    nc.tensor.matmul(pst[:ksz, :],
                                                     kTr_c[hp:hp + 64, k0:k0 + ksz],
                                                     qTr_c[hp:hp + 64, :],
                                                     start=True, stop=True)
                                    nc.scalar.activation(out=expT[:ksz, kc, :],
                                                         in_=pst[:ksz, :], func=AF.Exp,
                                                         scale=SCL)
                                # AV accumulate (per-head psum, partition base 0)
                                pav = ppA.tile([64, L], F32, tag="pav", bufs=2)
                                for kc, (k0, ksz) in enumerate(TCH):
                                    nc.tensor.matmul(pav,
                                                     V_r[:ksz, kc, h * 64:(h + 1) * 64],
                                                     expT[:ksz, kc, :],
                                                     start=(kc == 0), stop=(kc == 2))
                                # inv-l row -> replicated tile
                                ilrow = pah.tile([1, L], F32, tag="ilrow")
                                for (s0, ssz), src in (((0, 64), il64[:, h:h + 1]),
                                                       ((64, 128), il_r[:, 0, h:h + 1]),
                                                       ((192, 128), il_r[:, 1, h:h + 1])):
                                    pi = ppA.tile([1, 128], F32, tag="sm", bufs=1)
                                    nc.tensor.transpose(pi[:, :ssz], src,
                                                        ident[:ssz, :ssz])
                                    nc.vector.tensor_copy(out=ilrow[:, s0:s0 + ssz],
                                                          in_=pi[:, :ssz])
                                rep = pah.tile([64, L], F32, tag="rep")
                                nc.gpsimd.partition_broadcast(rep, ilrow)
                                # scale AV by inv-l, write avT rows (fp32r)
                                if hp == 0:
                                    nc.vector.tensor_tensor(out=avT_r[0:64, hc, :],
                                                            in0=pav, in1=rep,
                                                            op=OP.mult)
                                else:
                                    avtmp = pah.tile([64, L], F32R, tag="avtmp")
                                    nc.vector.tensor_tensor(out=avtmp, in0=pav,
                                                            in1=rep, op=OP.mult)
                                    nc.sync.dma_start(out=avT_r[64:128, hc, :],
                                                      in_=avtmp)
                        # ---- attn_t + argsort ranks ----
                        pat = ppA.tile([1, 256], F32, tag="sm", bufs=1)
                        nc.tensor.matmul(pat, ones64, acc, start=True, stop=True)
                        at_row = pa.tile([1, 256], F32, tag="at_row")
                        nc.vector.tensor_scalar(out=at_row, in0=pat, scalar1=INV768,
                                                scalar2=None, op0=OP.mult)
                        v_bcast = pa.tile([128, 256], F32, tag="v_bcast")
                        nc.gpsimd.partition_broadcast(v_bcast, at_row)
                        v_part = pa.tile([128, 2, 1], F32, tag="v_part")
                        for c in range(2):
                            pvp = ppA.tile([128, 1], F32, tag="sm", bufs=1)
                            nc.tensor.transpose(pvp, at_row[:, c * 128:(c + 1) * 128],
                                                ident[:1, :1])
                            nc.vector.tensor_copy(out=v_part[:, c, :], in_=pvp)
                        oh = pa.tile([128, 2, 256], F32R, tag="oh")
                        for c in range(2):
                            cmpt = pah.tile([128, 256], F32, tag="cmpt", bufs=1)
                            nc.vector.tensor_scalar(out=cmpt, in0=v_bcast,
                                                    scalar1=v_part[:, c, :], scalar2=None,
                                                    op0=OP.is_gt)
                            rk = pah.tile([128, 3], F32, tag="rk")
                            nc.vector.reduce_sum(out=rk[:, 0:1], in_=cmpt, axis=AX.X)
                            nc.vector.tensor_scalar(out=cmpt, in0=v_bcast,
                                                    scalar1=v_part[:, c, :], scalar2=None,
                                                    op0=OP.is_equal)
                            nc.gpsimd.affine_select(out=cmpt, in_=cmpt, compare_op=OP.is_gt,
                                                    fill=0.0, base=c * 128,
                                                    pattern=[[-1, 256]],
                                                    channel_multiplier=1)
                            nc.vector.reduce_sum(out=rk[:, 1:2], in_=cmpt, axis=AX.X)
                            nc.vector.tensor_tensor(out=rk[:, 2:3], in0=rk[:, 0:1],
                                                    in1=rk[:, 1:2], op=OP.add)
                            nc.vector.tensor_scalar(out=oh[:, c, :], in0=iota_f,
                                                    scalar1=rk[:, 2:3], scalar2=None,
                                                    op0=OP.is_equal)
                        # sorted global indices (keep + removed), fp32r exact
                        gp = pa.tile([128, 2, 8], F32, tag="gp")
                        nc.sync.dma_start(out=gp,
                                          in_=g_d.ap()[sb].rearrange("c p o -> p c o"))
                        gp_r = pa.tile([128, 2, 8], F32R, tag="gp_r")
                        nc.vector.tensor_copy(out=gp_r, in_=gp)
                        sg = pa.tile([128, 2, 1], F32, tag="sgt")
                        for rc in range(2):
                            psg = ppA.tile([128, 8], F32, tag="sm", bufs=1)
                            for ic in range(2):
                                nc.tensor.matmul(psg, oh[:, ic, rc * 128:(rc + 1) * 128],
                                                 gp_r[:, ic, :], start=(ic == 0),
                                                 stop=(ic == 1))
                            nc.vector.tensor_copy(out=sg[:, rc, :], in_=psg[:, 0:1])
                        nc.sync.dma_start(out=sg_d.ap()[sb].rearrange("c p o -> p c o"),
                                          in_=sg)
                        # ---- proj + residual -> xpa (fp32r) ----
                        xpa = pa.tile([128, 3, DIM], F32R, tag="Vr_xpa")
                        for c, (t0, tsz) in enumerate(TCH):
                            for nn in range(2):
                                pr = ppA.tile([128, 384], F32, tag="psv", bufs=2)
                                for kc in range(6):
                                    nc.tensor.matmul(pr[:tsz, :],
                                                     avT_r[:, kc, t0:t0 + tsz],
                                                     proj_r[:, kc, nn * 384:(nn + 1) * 384],
                                                     start=(kc == 0), stop=(kc == 5))
                                tmp = pah.tile([128, 384], F32, tag="prtmp")
                                nc.vector.tensor_tensor(
                                    out=tmp[:tsz], in0=pr[:tsz, :],
                                    in1=projb_rep[:tsz, nn * 384:(nn + 1) * 384],
                                    op=OP.add)
                                nc.vector.tensor_tensor(
                                    out=xpa[:tsz, c, nn * 384:(nn + 1) * 384],
                                    in0=tmp[:tsz],
                                    in1=xt[:tsz, c, nn * 384:(nn + 1) * 384], op=OP.add)
                        # spill rows 0..64 (template tokens)
                        nc.sync.dma_start(out=xsp_d[sb, 0:LT, :],
                                          in_=xpa[0:64, 0, :].bitcast(F32))
                        # re-align search rows 64..320 via sbuf->sbuf DMA
                        xs_al = pa.tile([128, 2, DIM], F32R, tag="xs_al")
                        nc.sync.dma_start(out=xs_al[0:64, 0, :], in_=xpa[64:128, 0, :])
                        nc.sync.dma_start(out=xs_al[64:128, 0, :], in_=xpa[0:64, 1, :])
                        nc.sync.dma_start(out=xs_al[0:64, 1, :], in_=xpa[64:128, 1, :])
                        nc.sync.dma_start(out=xs_al[64:128, 1, :], in_=xpa[0:64, 2, :])
                        # gather attentive tokens by rank (one-hot matmul)
                        for rc, msz in ((0, 128), (1, 52)):
                            xg = pah.tile([128, DIM], F32, tag="xg", bufs=1)
                            for nn in range(2):
                                pg = ppA.tile([128, 384], F32, tag="psv", bufs=2)
                                for ic in range(2):
                                    nc.tensor.matmul(
                                        pg[:msz, :],
                                        oh[:, ic, rc * 128:rc * 128 + msz],
                                        xs_al[:, ic, nn * 384:(nn + 1) * 384],
                                        start=(ic == 0), stop=(ic == 1))
                                nc.scalar.copy(out=xg[:msz, nn * 384:(nn + 1) * 384],
                                               in_=pg[:msz, :])
                            nc.sync.dma_start(
                                out=xsp_d[sb, LT + rc * 128:LT + rc * 128 + msz, :],
                                in_=xg[:msz, :])

            # ================= phase B: MLP ==================================
            with tc.tile_pool(name="wb", bufs=1) as wb:
                W1_r = wb.tile([128, 6, HID], F32R)
                W2_r = wb.tile([128, 24, DIM], F32R)
                b2_col = wb.tile([128, 24, 1], F32)
                b3_rep = wb.tile([128, DIM], F32)

                with (
                    tc.tile_pool(name="wtmp2", bufs=2) as wt2_pool,
                    tc.tile_pool(name="ppW2", bufs=1, space="PSUM") as ppW2,
                ):
                    fc1b_row = wt2_pool.tile([1, HID], F32, tag="b1row", bufs=1)
                    nc.sync.dma_start(out=fc1b_row, in_=fc1b_d.ap())
                    fc2b_row = wt2_pool.tile([1, DIM], F32, tag="b2row", bufs=1)
                    nc.sync.dma_start(out=fc2b_row, in_=fc2b_d.ap())
                    nc.gpsimd.partition_broadcast(b3_rep, fc2b_row)
                    n2b_cr = wt2_pool.tile([128, 6, 1], F32R, tag="n2br", bufs=1)
                    nc.vector.tensor_copy(out=n2b_cr, in_=n2b_c)

                    for c in range(6):
                        w1t = wt2_pool.tile([128, HID], F32, tag="w1t")
                        nc.sync.dma_start(out=w1t,
                                          in_=fc1w_d.ap()[c * 128:(c + 1) * 128, :])
                        nc.vector.tensor_scalar(out=W1_r[:, c, :], in0=w1t,
                                                scalar1=n2w_c[:, c, :], scalar2=None,
                                                op0=OP.mult)
                    beff_row = wt2_pool.tile([1, HID], F32, tag="beff", bufs=1)
                    for nn in range(8):
                        pbx = ppW2.tile([1, 384], F32, tag="pb2", bufs=2)
                        for c in range(6):
                            nc.tensor.matmul(pbx, n2b_cr[:, c, :],
                                             W1_r[:, c, nn * 384:(nn + 1) * 384],
                                             start=(c == 0), stop=(c == 5))
                        nc.vector.tensor_tensor(out=beff_row[:, nn * 384:(nn + 1) * 384],
                                                in0=pbx,
                                                in1=fc1b_row[:, nn * 384:(nn + 1) * 384],
                                                op=OP.add)
                    for c in range(24):
                        pbx2 = ppW2.tile([128, 1], F32, tag="pbt2", bufs=2)
                        nc.tensor.transpose(pbx2, beff_row[:, c * 128:(c + 1) * 128],
                                            ident[:1, :1])
                        nc.vector.tensor_copy(out=b2_col[:, c, :], in_=pbx2)
                        w2t = wt2_pool.tile([128, DIM], F32, tag="w2t")
                        nc.sync.dma_start(out=w2t,
                                          in_=fc2w_d.ap()[c * 128:(c + 1) * 128, :])
                        nc.vector.tensor_copy(out=W2_r[:, c, :], in_=w2t)

                with (
                    tc.tile_pool(name="pb", bufs=1) as pb,
                    tc.tile_pool(name="pbh", bufs=2) as pbh,
                    tc.tile_pool(name="ppB", bufs=1, space="PSUM") as ppB,
                ):
                    for sb in range(NSB):
                        xn = pb.tile([128, 2, DIM], F32, tag="xn", bufs=2)
                        for c, (t0, tsz) in enumerate(OCH):
                            nc.sync.dma_start(out=xn[:tsz, c, :],
                                              in_=xsp_d[sb, t0:t0 + tsz, :])
                        # LN2 (plain rstd; MLP path is tolerance-level)
                        ln2 = pb.tile([128, 2, DIM], F32, tag="ln2")
                        for c, (t0, tsz) in enumerate(OCH):
                            st = pbh.tile([128, 3, 6], F32, tag="bnst2")
                            for g in range(3):
                                nc.vector.bn_stats(out=st[:tsz, g, :],
                                                   in_=xn[:tsz, c, g * 256:(g + 1) * 256])
                            mv = pbh.tile([128, 2], F32, tag="bnmv2")
                            nc.vector.bn_aggr(out=mv[:tsz], in_=st[:tsz])
                            sd = pbh.tile([128, 2], F32, tag="sd2")
                            nc.scalar.activation(out=sd[:tsz, 0:1], in_=mv[:tsz, 1:2],
                                                 func=AF.Sqrt, bias=eps128[:tsz],
                                                 scale=1.0)
                            nc.vector.reciprocal(out=sd[:tsz, 1:2], in_=sd[:tsz, 0:1])
                            nc.vector.tensor_scalar(out=ln2[:tsz, c, :],
                                                    in0=xn[:tsz, c, :],
                                                    scalar1=mv[:tsz, 0:1],
                                                    scalar2=sd[:tsz, 1:2],
                                                    op0=OP.subtract, op1=OP.mult)
                        # ln2T padded to 256 cols (fp32r fast path needs n>=256)
                        ln2T_r = pb.tile([128, 6, 256], F32R, tag="ln2T")
                        nc.vector.memset(ln2T_r[:, :, LOUT:256].bitcast(F32), 0.0)
                        for dc in range(6):
                            pt = ppB.tile([128, 244], F32, tag="pst", bufs=2)
                            for c, (t0, tsz) in enumerate(OCH):
                                nc.tensor.matmul(pt[:, t0:t0 + tsz],
                                                 ln2[:tsz, c, dc * 128:(dc + 1) * 128],
                                                 ident[:tsz, :tsz], is_transpose=True,
                                                 skip_group_check=True)
                            nc.vector.tensor_copy(out=ln2T_r[:, dc, 0:LOUT], in_=pt)
                        # fc1 -> gelu -> fc2 (h chunk-streamed)
                        px = [[ppB.tile([128, 384], F32, tag="psx", bufs=4, name=f"px{a}{b}")
                               for b in range(2)] for a in range(2)]
                        for fc in range(24):
                            ph = ppB.tile([128, 256], F32, tag="psh", bufs=2)
                            for kc in range(6):
                                nc.tensor.matmul(ph,
                                                 W1_r[:, kc, fc * 128:(fc + 1) * 128],
                                                 ln2T_r[:, kc, :], start=(kc == 0),
                                                 stop=(kc == 5))
                            hT = pbh.tile([128, LOUT], F32R, tag="hT")
                            nc.scalar.activation(out=hT, in_=ph[:, 0:LOUT], func=AF.Gelu,
                                                 bias=b2_col[:, fc, :], scale=1.0)
                            for tc_i, (t0, tsz) in enumerate(OCH):
                                for nn in range(2):
                                    nc.tensor.matmul(px[tc_i][nn][:tsz, :],
                                                     hT[:, t0:t0 + tsz],
                                                     W2_r[:, fc, nn * 384:(nn + 1) * 384],
                                                     start=(fc == 0), stop=(fc == 23))
                        for tc_i, (t0, tsz) in enumerate(OCH):
                            for nn in range(2):
                                tmp = pbh.tile([128, 384], F32, tag="mtmp")
                                nc.vector.tensor_tensor(
                                    out=tmp[:tsz], in0=px[tc_i][nn][:tsz, :],
                                    in1=b3_rep[:tsz, nn * 384:(nn + 1) * 384], op=OP.add)
                                nc.vector.tensor_tensor(
                                    out=xn[:tsz, tc_i, nn * 384:(nn + 1) * 384],
                                    in0=xn[:tsz, tc_i, nn * 384:(nn + 1) * 384],
                                    in1=tmp[:tsz], op=OP.add)
                        for c, (t0, tsz) in enumerate(OCH):
                            nc.sync.dma_start(out=xo_d.ap()[sb, t0:t0 + tsz, :],
                                              in_=xn[:tsz, c, :])

    nc.compile()
    return nc


def _get_program():
    if "nc" not in _CACHE:
        _CACHE["nc"] = _build_program()
    return _CACHE["nc"]


def kernel(**inputs):
    from concourse.bass_utils import run_bass_kernel_spmd

    nc = _get_program()

    x = np.asarray(inputs["x"], dtype=np.float32)
    xi = np.asarray(inputs["xi"], dtype=np.float32)
    gs = np.asarray(inputs["global_index_search"])
    gsi = np.asarray(inputs["global_index_searchi"])
    gs_f = np.repeat(gs.astype(np.float32).reshape(B, 2, 128, 1), 8, axis=3)
    gsi_f = np.repeat(gsi.astype(np.float32).reshape(B, 2, 128, 1), 8, axis=3)

    weights = {
        "qkv_w": np.ascontiguousarray(inputs["qkv_w"], np.float32),
        "qkv_b": np.ascontiguousarray(inputs["qkv_b"], np.float32).reshape(1, -1),
        "n1w": np.ascontiguousarray(inputs["norm1_w"], np.float32).reshape(1, -1),
        "n1b": np.ascontiguousarray(inputs["norm1_b"], np.float32).reshape(1, -1),
        "proj_w": np.ascontiguousarray(inputs["proj_w"], np.float32),
        "proj_b": np.ascontiguousarray(inputs["proj_b"], np.float32).reshape(1, -1),
        "n2w": np.ascontiguousarray(inputs["norm2_w"], np.float32).reshape(1, -1),
        "n2b": np.ascontiguousarray(inputs["norm2_b"], np.float32).reshape(1, -1),
        "fc1_w": np.ascontiguousarray(inputs["fc1_w"], np.float32),
        "fc1_b": np.ascontiguousarray(inputs["fc1_b"], np.float32).reshape(1, -1),
        "fc2_w": np.ascontiguousarray(inputs["fc2_w"], np.float32),
        "fc2_b": np.ascontiguousarray(inputs["fc2_b"], np.float32).reshape(1, -1),
    }

    in_maps = []
    for core in range(NCORES):
        b0 = core * BSH
        xs = np.concatenate([x[b0:b0 + BSH], xi[b0:b0 + BSH]], axis=0)
        gcat = np.concatenate([gs_f[b0:b0 + BSH], gsi_f[b0:b0 + BSH]], axis=0)
        in_maps.append({"xs": np.ascontiguousarray(xs),
                        "gidx": np.ascontiguousarray(gcat), **weights})

    res = run_bass_kernel_spmd(nc, in_maps, list(range(NCORES)))

    attn = np.empty((B, HEADS, L, L), np.float32)
    i_attn = np.empty((B, HEADS, L, L), np.float32)
    xo = np.empty((B, LOUT, DIM), np.float32)
    xio = np.empty((B, LOUT, DIM), np.float32)
    sg = np.empty((B, 256), np.float32)
    sgi = np.empty((B, 256), np.float32)
    for core in range(NCORES):
        r = res.results[core]
        b0 = core * BSH
        attn[b0:b0 + BSH] = r["attn"][:BSH]
        i_attn[b0:b0 + BSH] = r["attn"][BSH:]
        xo[b0:b0 + BSH] = r["xo"][:BSH]
        xio[b0:b0 + BSH] = r["xo"][BSH:]
        sg[b0:b0 + BSH] = r["sg"][:BSH].reshape(BSH, 256)
        sgi[b0:b0 + BSH] = r["sg"][BSH:].reshape(BSH, 256)

    keep_s = np.rint(sg[:, :KEEP]).astype(gs.dtype)
    removed_s = np.rint(sg[:, KEEP:]).astype(gs.dtype)
    keep_si = np.rint(sgi[:, :KEEP]).astype(gsi.dtype)
    removed_si = np.rint(sgi[:, KEEP:]).astype(gsi.dtype)

    git = np.asarray(inputs["global_index_template"]).copy()
    giti = np.asarray(inputs["global_index_templatei"]).copy()
    return (xo, git, keep_s, removed_s, attn,
            xio, giti, keep_si, removed_si, i_attn)
